# revision 13
# baseline (speedup 1.0000x reference)
"""Multi-head attention (B=8, S=1024, D=2048, H=16) on 8 Trainium2 NeuronCores.

Sharding: pure data parallel — core b computes batch element b. Weights are
replicated. All matmuls in bf16 (1 cyc/row on PE, same as f32r, but with FWL
weight loads and half the DMA/SBUF), accumulation in fp32 PSUM.

Structure per core:
  - q/k/v are transposed by the DMA xbar (dma_start_transpose) straight into
    SBUF as xT chunk tiles [128, S] — zero PE transpose cost.
  - V = v@Wv staged in SBUF augmented with a ones column per chunk:
    V_aug[:, c, h, 0:128] = V chunk, V_aug[:, c, h, 128] = 1.
  - Per head h: project QT_h/KT_h (W chunk stationary, xT moving), scores
    (KT chunk stationary), exp on ACT -> e chunks [sk, sq] bf16, then PV with
    the e chunks STATIONARY and V_aug moving:
        out[sq, 0:129] = sum_c e_c[:, sq-tile]^T @ V_aug[:, c, h, :]
    Column 128 is the softmax denominator in natural per-partition layout, so
    normalization is one reciprocal + one per-partition tensor_scalar mul.
    No tail transposes, no separate denominator matmuls.
  - PV(h) matmuls are interleaved into the scores(h+1) stream so the PE fills
    the gaps where scores wait on ACT draining PSUM.

Self-contained: builds the Bass program, shards inputs, runs SPMD via PJRT,
reassembles the full output.
"""
import numpy as np
from contextlib import ExitStack

import concourse.bacc as bacc
import concourse.mybir as mybir
import concourse.tile as tile

B, S, D, H = 8, 1024, 2048, 16
DH = D // H            # 128
NK = D // 128          # 16 k-chunks
NS = S // 128          # 8 s-tiles
F32 = mybir.dt.float32
BF16 = mybir.dt.bfloat16
SCALE = 1.0 / float(np.sqrt(DH))

_CACHE = {}


def build(opt=None, reps=1, timing=False):
    _defaults = dict(
        ps1024_bufs=3,
        ps512_bufs=2,
        e_bufs=2,
        w_bufs=2,
        qk_bufs=2,
        osb_bufs=2,
        sc_pv_pattern=1,   # interleave PV tiles between scores chunks
    )
    _defaults.update(opt or {})
    opt = _defaults
    nc = bacc.Bacc("TRN2", target_bir_lowering=False, debug=False)

    def _in(name, shape, dt_):
        if timing:
            return nc.dram_tensor(name, shape, dt_).ap()
        return nc.dram_tensor(name, shape, dt_, kind="ExternalInput").ap()

    q_d = _in("q", [S, D], BF16)
    k_d = _in("k", [S, D], BF16)
    v_d = _in("v", [S, D], BF16)
    wq_d = _in("Wq", [D, D], BF16)
    wk_d = _in("Wk", [D, D], BF16)
    wv_d = _in("Wv", [D, D], BF16)
    if timing:
        out_d = nc.dram_tensor("out", [S, D], F32).ap()
        tout_d = nc.dram_tensor("tout", [1, 8], F32, kind="ExternalOutput").ap()
    else:
        out_d = nc.dram_tensor("out", [S, D], F32, kind="ExternalOutput").ap()

    with tile.TileContext(nc) as tc, ExitStack() as ctx:
        ps1024 = ctx.enter_context(
            tc.tile_pool(name="ps1024", bufs=opt["ps1024_bufs"], space="PSUM"))
        ps512 = ctx.enter_context(
            tc.tile_pool(name="ps512", bufs=opt["ps512_bufs"], space="PSUM"))
        const = ctx.enter_context(tc.tile_pool(name="const", bufs=1))

        ones_f = const.tile([128, 16, 1], F32, name="ones_f")
        nc.gpsimd.memset(ones_f[:], 1.0)
        ones_bf = const.tile([128, 16, 1], BF16, name="ones_bf")
        nc.vector.tensor_copy(ones_bf[:], ones_f[:])

        if timing:
            with tc.tile_pool(name="zfill", bufs=1) as zpool:
                zf = zpool.tile([128, D], F32, name="zfill")
                nc.gpsimd.memset(zf[:], 0.0)
                zr = zpool.tile([128, D], BF16, name="zfill_b")
                nc.vector.tensor_copy(zr[:], zf[:])
                for x in (q_d, k_d, v_d, wq_d, wk_d, wv_d):
                    nrow = x.shape[0] // 128
                    for i in range(nrow):
                        nc.sync.dma_start(x[i * 128:(i + 1) * 128, :], zr[:])

        for _rep in range(reps):
            _body_once(nc, tc, ps1024, ps512, ones_bf,
                       q_d, k_d, v_d, wq_d, wk_d, wv_d, out_d, opt)
        if timing:
            zo = const.tile([1, 8], F32, name="zo")
            nc.gpsimd.memset(zo[:], 0.0)
            nc.sync.dma_start(tout_d[:], zo[:])

    nc.compile()
    return nc


def _body_once(nc, tc, ps1024, ps512, ones_bf,
               q_d, k_d, v_d, wq_d, wk_d, wv_d, out_d, opt):
    with ExitStack() as body:
        pool_qT = body.enter_context(tc.tile_pool(name="qT", bufs=1))
        pool_kT = body.enter_context(tc.tile_pool(name="kT", bufs=1))
        pool_vaug = body.enter_context(tc.tile_pool(name="vaug", bufs=1))

        qT = [pool_qT.tile([128, S], BF16, name=f"qT{j}") for j in range(NK)]
        kT = [pool_kT.tile([128, S], BF16, name=f"kT{j}") for j in range(NK)]
        # V_aug[p, c, h, 0:128] = V[c*128+p, h*128+j]; [..., 128] = 1.0
        vaug = pool_vaug.tile([128, NS, H, DH + 1], BF16, name="vaug")

        pool_w = body.enter_context(tc.tile_pool(name="wqk", bufs=opt["w_bufs"]))

        def _load_w(h):
            tiles = {}
            for nm, w_d in (("wq", wq_d), ("wk", wk_d)):
                t = pool_w.tile([128, NK, DH], BF16, name=nm)
                nc.scalar.dma_start(
                    t[:], w_d[:, h * DH:(h + 1) * DH].rearrange(
                        "(kc p) n -> p kc n", p=128))
                tiles[nm] = t
            return tiles

        # ---------------- Phase V: v -> vT -> V -> V_aug (SBUF) -------------
        with ExitStack() as pv:
            pool_vT = pv.enter_context(tc.tile_pool(name="vT", bufs=1))
            pool_wv = pv.enter_context(tc.tile_pool(name="wv", bufs=opt["w_bufs"]))

            vT = [pool_vT.tile([128, S], BF16, name=f"vT{j}") for j in range(NK)]
            for j in range(NK):
                nc.sync.dma_start_transpose(vT[j][:], v_d[:, j * 128:(j + 1) * 128])

            def _load_wv(n):
                t = pool_wv.tile([128, NK, 512], BF16, name="wv")
                src = wv_d[:, n * 512:(n + 1) * 512].rearrange(
                    "(kc p) n -> p kc n", p=128)
                for half in range(2):
                    nq = NK // 2
                    nc.scalar.dma_start(t[:, half * nq:(half + 1) * nq, :],
                                        src[:, half * nq:(half + 1) * nq, :])
                return t

            # prefetch wv0/wv1 and the first heads' W slices ahead of the
            # q/k transposes so the V pipeline is never DMA-starved
            wv_pre = [_load_wv(0), _load_wv(1)]
            w_tiles = _load_w(0)
            w_next = _load_w(1)

            # q/k transposes ride the same queue; they complete during V compute
            for j in range(NK):
                nc.sync.dma_start_transpose(qT[j][:], q_d[:, j * 128:(j + 1) * 128])
            for j in range(NK):
                nc.sync.dma_start_transpose(kT[j][:], k_d[:, j * 128:(j + 1) * 128])

            # ones column of V_aug
            for m in range(NS):
                nc.vector.tensor_copy(vaug[:, m, :, DH:DH + 1], ones_bf[:])

            # V projection, n-slice outer so wv double-buffers cleanly
            for n in range(4):
                wv_t = wv_pre[n] if n < 2 else _load_wv(n)
                for m in range(NS):
                    ps = ps512.tile([128, 512], F32, name="ps512")
                    for jk in range(NK):
                        nc.tensor.matmul(
                            ps[:], vT[jk][:, m * 128:(m + 1) * 128],
                            wv_t[:, jk, :],
                            start=(jk == 0), stop=(jk == NK - 1))
                    # evacuate into V_aug (dest strided over the head dim)
                    nc.vector.tensor_copy(
                        vaug[:, m, n * 4:(n + 1) * 4, 0:DH],
                        ps[:].rearrange("p (h d) -> p h d", d=DH))

        # ---------------- Per-head pipeline ----------------
        with ExitStack() as ph:
            pool_hh = ph.enter_context(tc.tile_pool(name="hh", bufs=opt["qk_bufs"]))
            pool_e = ph.enter_context(tc.tile_pool(name="e", bufs=opt["e_bufs"]))
            pool_rs = ph.enter_context(tc.tile_pool(name="rs", bufs=opt["osb_bufs"]))
            pool_osb = ph.enter_context(tc.tile_pool(name="osb", bufs=opt["osb_bufs"]))

            def _proj_mms(ps, w_tiles, nm, j0, j1):
                xT = qT if nm == "wq" else kT
                for jk in range(j0, j1):
                    for half in range(2):
                        nc.tensor.matmul(
                            ps[:, half * 512:(half + 1) * 512],
                            w_tiles[nm][:, jk, :],
                            xT[jk][:, half * 512:(half + 1) * 512],
                            start=(jk == 0), stop=(jk == NK - 1))

            def _proj_evac(ps, nm):
                o = pool_hh.tile([128, S], BF16, name=f"hh_{nm}")
                # two half evacuations: the first scores chunk only waits on
                # the first half instead of the whole [128, 1024] copy
                nc.vector.tensor_copy(o[:, 0:512], ps[:, 0:512])
                nc.vector.tensor_copy(o[:, 512:1024], ps[:, 512:1024])
                return o

            def _proj_one(w_tiles, nm):
                """QT_h or KT_h [128(dh), S] bf16 (32 MMs + 2 half evacs)."""
                ps = ps1024.tile([128, 1024], F32, name="ps1024")
                _proj_mms(ps, w_tiles, nm, 0, NK)
                return _proj_evac(ps, nm)

            def _score_chunk(qh, kh, e_tiles, c):
                ps = ps1024.tile([128, 1024], F32, name="ps1024")
                for half in range(2):
                    nc.tensor.matmul(
                        ps[:, half * 512:(half + 1) * 512],
                        kh[:, c * 128:(c + 1) * 128],
                        qh[:, half * 512:(half + 1) * 512],
                        start=True, stop=True)
                e_t = pool_e.tile([128, 1024], BF16, name=f"e{c}")
                nc.scalar.activation(e_t[:], ps[:],
                                     mybir.ActivationFunctionType.Exp,
                                     scale=SCALE)
                e_tiles[c] = e_t

            def _pv_tile(h, e_tiles, o_sb, t):
                ps = ps512.tile([128, 512], F32, name="ps512")
                po = ps[:, 0:DH + 1]
                for c in range(NS):
                    nc.tensor.matmul(po, e_tiles[c][:, t * 128:(t + 1) * 128],
                                     vaug[:, c, h, :],
                                     start=(c == 0), stop=(c == NS - 1))
                rs = pool_rs.tile([128, 1], F32, name="rs")
                nc.vector.reciprocal_approx_fast(rs[:], ps[:, DH:DH + 1])
                nc.vector.tensor_scalar_mul(o_sb[:, t, :], ps[:, 0:DH], rs[:])

            def _store_head(h, o_sb):
                nc.gpsimd.dma_start(
                    out_d[:, h * DH:(h + 1) * DH].rearrange(
                        "(t p) d -> p t d", p=128),
                    o_sb[:])

            # Shifted pipeline: iteration h runs PV(h) interleaved with
            # scores(h+1), then emits proj(h+2) as ACT-hiding filler.
            # Prologue: proj(0) + proj(1), with scores(0) spaced by proj(1)
            # pieces so ACT exp keeps up.
            q0 = _proj_one(w_tiles, "wq")
            k0 = _proj_one(w_tiles, "wk")
            e_cur = [None] * NS
            _score_chunk(q0, k0, e_cur, 0)
            _score_chunk(q0, k0, e_cur, 1)
            q1 = _proj_one(w_next, "wq")
            _score_chunk(q0, k0, e_cur, 2)
            _score_chunk(q0, k0, e_cur, 3)
            k1 = _proj_one(w_next, "wk")
            for c in range(4, NS):
                _score_chunk(q0, k0, e_cur, c)
            cur = (q1, k1)
            w_next = _load_w(2)

            for h in range(H):
                o_sb = pool_osb.tile([128, NS, DH], F32, name="osb")
                e_nxt = [None] * NS
                if h + 1 < H:
                    # 3 PV tiles up front cover the QT/KT evac latency of the
                    # upcoming scores; the next 5 interleave 1:1; the last two
                    # scores chunks weave between proj(h+2) halves so ACT exp
                    # never gates the PE
                    _pv_tile(h, e_cur, o_sb, 0)
                    _pv_tile(h, e_cur, o_sb, 1)
                    _pv_tile(h, e_cur, o_sb, 2)
                    for c in range(NS - 2):
                        _score_chunk(cur[0], cur[1], e_nxt, c)
                        if 3 + c < NS:
                            _pv_tile(h, e_cur, o_sb, 3 + c)
                    if h + 2 < H:
                        ps_q = ps1024.tile([128, 1024], F32, name="ps1024")
                        _proj_mms(ps_q, w_next, "wq", 0, NK)
                        nq = _proj_evac(ps_q, "wq")
                        _score_chunk(cur[0], cur[1], e_nxt, NS - 2)
                        ps_k = ps1024.tile([128, 1024], F32, name="ps1024")
                        _proj_mms(ps_k, w_next, "wk", 0, NK // 2)
                        _score_chunk(cur[0], cur[1], e_nxt, NS - 1)
                        _proj_mms(ps_k, w_next, "wk", NK // 2, NK)
                        nk = _proj_evac(ps_k, "wk")
                        nxt = (nq, nk)
                        w_next = _load_w(h + 3) if h + 3 < H else None
                    else:
                        _score_chunk(cur[0], cur[1], e_nxt, NS - 2)
                        _score_chunk(cur[0], cur[1], e_nxt, NS - 1)
                        nxt = None
                else:
                    for t in range(NS):
                        _pv_tile(h, e_cur, o_sb, t)
                    nxt = None
                _store_head(h, o_sb)
                cur, e_cur = nxt if nxt is not None else cur, e_nxt


def _make_runner(nc, n_cores):
    """Jitted SPMD runner (q/k/v sharded over cores, weights replicated)."""
    import jax
    from jax.sharding import Mesh, PartitionSpec
    from jax.experimental.shard_map import shard_map
    from concourse import bass2jax
    from concourse.bass2jax import _bass_exec_p, install_neuronx_cc_hook

    install_neuronx_cc_hook()
    partition_name = nc.partition_id_tensor.name if nc.partition_id_tensor else None
    in_names, out_names, out_avals, zero_outs = [], [], [], []
    for alloc in nc.m.functions[0].allocations:
        if not isinstance(alloc, mybir.MemoryLocationSet):
            continue
        name = alloc.memorylocations[0].name
        if alloc.kind == "ExternalInput":
            if name != partition_name:
                in_names.append(name)
        elif alloc.kind == "ExternalOutput":
            out_names.append(name)
            shape = tuple(alloc.tensor_shape)
            dtype = mybir.dt.np(alloc.dtype)
            out_avals.append(jax.core.ShapedArray(shape, dtype))
            zero_outs.append(np.zeros(shape, dtype))
    sharded_in = {"q", "k", "v"}
    in_names_all = in_names + out_names
    if partition_name is not None:
        in_names_all.append(partition_name)

    def _body(*args):
        operands = list(args)
        if partition_name is not None:
            operands.append(bass2jax.partition_id_tensor())
        outs = _bass_exec_p.bind(
            *operands,
            out_avals=tuple(out_avals),
            in_names=tuple(in_names_all),
            out_names=tuple(out_names),
            lowering_input_output_aliases=(),
            sim_require_finite=True,
            sim_require_nnan=True,
            nc=nc,
        )
        return tuple(outs)

    devices = jax.devices()[:n_cores]
    mesh = Mesh(np.asarray(devices), ("core",))
    in_specs = tuple(
        PartitionSpec("core") if n in sharded_in else PartitionSpec()
        for n in in_names
    ) + (PartitionSpec("core"),) * len(out_names)
    out_specs = (PartitionSpec("core"),) * len(out_names)
    jitted = jax.jit(
        shard_map(_body, mesh=mesh, in_specs=in_specs, out_specs=out_specs,
                  check_rep=False),
        keep_unused=True,
    )

    def run(shared_map_, per_core_maps):
        import jax as _jax
        args = []
        for n in in_names:
            if n in sharded_in:
                args.append(np.concatenate([m[n] for m in per_core_maps], axis=0))
            else:
                args.append(shared_map_[n])
        concat_zeros = [
            np.zeros((n_cores * z.shape[0], *z.shape[1:]), z.dtype) for z in zero_outs
        ]
        out_arrs = jitted(*args, *concat_zeros)
        _jax.block_until_ready(out_arrs)
        return [
            {
                name: np.asarray(out_arrs[i]).reshape(n_cores, *out_avals[i].shape)[c]
                for i, name in enumerate(out_names)
            }
            for c in range(n_cores)
        ]

    return run


def _to_bf16(x):
    import ml_dtypes
    return np.asarray(x, dtype=np.float32).astype(ml_dtypes.bfloat16)


def _get_compiled():
    if "run" not in _CACHE:
        nc = build()
        _CACHE["run"] = _make_runner(nc, B)
    return _CACHE["run"]


def kernel(q, k, v, Wq, Wk, Wv):
    run = _get_compiled()
    shared = {"Wq": _to_bf16(Wq), "Wk": _to_bf16(Wk), "Wv": _to_bf16(Wv)}
    q = _to_bf16(q)
    k = _to_bf16(k)
    v = _to_bf16(v)
    per_core = [{"q": q[b], "k": k[b], "v": v[b]} for b in range(B)]
    results = run(shared, per_core)
    out = np.stack([results[b]["out"] for b in range(B)], axis=0)
    return out.astype(np.float32)


if __name__ == "__main__":
    rng = np.random.default_rng(0)
    qq = rng.standard_normal((B, S, D), dtype=np.float32)
    kk = rng.standard_normal((B, S, D), dtype=np.float32)
    vv = rng.standard_normal((B, S, D), dtype=np.float32)
    sc = np.float32(1.0 / np.sqrt(D))
    Wq = rng.standard_normal((D, D), dtype=np.float32) * sc
    Wk = rng.standard_normal((D, D), dtype=np.float32) * sc
    Wv = rng.standard_normal((D, D), dtype=np.float32) * sc
    o = kernel(q=qq, k=kk, v=vv, Wq=Wq, Wk=Wk, Wv=Wv)
    print("out", o.shape, o.dtype, np.abs(o).max())


# revision 20
# speedup vs baseline: 1.0043x; 1.0043x over previous
"""Multi-head attention (B=8, S=1024, D=2048, H=16) on 8 Trainium2 NeuronCores.

Sharding: pure data parallel — core b computes batch element b. Weights are
replicated. All matmuls in bf16 (1 cyc/row on PE, same as f32r, but with FWL
weight loads and half the DMA/SBUF), accumulation in fp32 PSUM.

Structure per core:
  - q/k/v are transposed by the DMA xbar (dma_start_transpose) straight into
    SBUF as xT chunk tiles [128, S] — zero PE transpose cost.
  - V = v@Wv staged in SBUF augmented with a ones column per chunk:
    V_aug[:, c, h, 0:128] = V chunk, V_aug[:, c, h, 128] = 1.
  - Per head h: project QT_h/KT_h (W chunk stationary, xT moving), scores
    (KT chunk stationary), exp on ACT -> e chunks [sk, sq] bf16, then PV with
    the e chunks STATIONARY and V_aug moving:
        out[sq, 0:129] = sum_c e_c[:, sq-tile]^T @ V_aug[:, c, h, :]
    Column 128 is the softmax denominator in natural per-partition layout, so
    normalization is one reciprocal + one per-partition tensor_scalar mul.
    No tail transposes, no separate denominator matmuls.
  - PV(h) matmuls are interleaved into the scores(h+1) stream so the PE fills
    the gaps where scores wait on ACT draining PSUM.

Self-contained: builds the Bass program, shards inputs, runs SPMD via PJRT,
reassembles the full output.
"""
import numpy as np
from contextlib import ExitStack

import concourse.bacc as bacc
import concourse.mybir as mybir
import concourse.tile as tile

B, S, D, H = 8, 1024, 2048, 16
DH = D // H            # 128
NK = D // 128          # 16 k-chunks
NS = S // 128          # 8 s-tiles
F32 = mybir.dt.float32
BF16 = mybir.dt.bfloat16
SCALE = 1.0 / float(np.sqrt(DH))

_CACHE = {}


def build(opt=None, reps=1, timing=False):
    _defaults = dict(
        ps1024_bufs=3,
        ps512_bufs=2,
        e_bufs=2,
        w_bufs=2,
        qk_bufs=2,
        osb_bufs=2,
        sc_pv_pattern=1,   # interleave PV tiles between scores chunks
        pv_mode="v_stat",  # "e_stat": e-chunk stationary, den via ones column
                           # "v_stat": V-chunk stationary (N=512 moving), den
                           #   via gpsimd E_sum + ones-matmul, out^T via xbar
    )
    _defaults.update(opt or {})
    opt = _defaults
    nc = bacc.Bacc("TRN2", target_bir_lowering=False, debug=False)

    def _in(name, shape, dt_):
        if timing:
            return nc.dram_tensor(name, shape, dt_).ap()
        return nc.dram_tensor(name, shape, dt_, kind="ExternalInput").ap()

    q_d = _in("q", [S, D], BF16)
    k_d = _in("k", [S, D], BF16)
    v_d = _in("v", [S, D], BF16)
    wq_d = _in("Wq", [D, D], BF16)
    wk_d = _in("Wk", [D, D], BF16)
    wv_d = _in("Wv", [D, D], BF16)
    if timing:
        out_d = nc.dram_tensor("out", [S, D], F32).ap()
        tout_d = nc.dram_tensor("tout", [1, 8], F32, kind="ExternalOutput").ap()
    else:
        out_d = nc.dram_tensor("out", [S, D], F32, kind="ExternalOutput").ap()

    with tile.TileContext(nc) as tc, ExitStack() as ctx:
        ps1024 = ctx.enter_context(
            tc.tile_pool(name="ps1024", bufs=opt["ps1024_bufs"], space="PSUM"))
        ps512 = ctx.enter_context(
            tc.tile_pool(name="ps512", bufs=opt["ps512_bufs"], space="PSUM"))
        const = ctx.enter_context(tc.tile_pool(name="const", bufs=1))

        ones_f = const.tile([128, 16], F32, name="ones_f")
        nc.gpsimd.memset(ones_f[:], 1.0)
        zeros_f = const.tile([128, 16, 3], F32, name="zeros_f")
        nc.gpsimd.memset(zeros_f[:], 0.0)
        ones_pad = const.tile([128, 16, 4], BF16, name="ones_pad")
        # col 0 = 1.0 (the den ones column), cols 1-3 = alignment pad zeros;
        # memset on a strided view is unreliable, so build via two copies
        nc.vector.tensor_copy(ones_pad[:, :, 0:1], ones_f[:])
        nc.vector.tensor_copy(ones_pad[:, :, 1:4], zeros_f[:])

        if timing:
            with tc.tile_pool(name="zfill", bufs=1) as zpool:
                zf = zpool.tile([128, D], F32, name="zfill")
                nc.gpsimd.memset(zf[:], 0.0)
                zr = zpool.tile([128, D], BF16, name="zfill_b")
                nc.vector.tensor_copy(zr[:], zf[:])
                for x in (q_d, k_d, v_d, wq_d, wk_d, wv_d):
                    nrow = x.shape[0] // 128
                    for i in range(nrow):
                        nc.sync.dma_start(x[i * 128:(i + 1) * 128, :], zr[:])

        for _rep in range(reps):
            _body_once(nc, tc, ps1024, ps512, ones_pad,
                       q_d, k_d, v_d, wq_d, wk_d, wv_d, out_d, opt)
        if timing:
            zo = const.tile([1, 8], F32, name="zo")
            nc.gpsimd.memset(zo[:], 0.0)
            nc.sync.dma_start(tout_d[:], zo[:])

    nc.compile()
    return nc


def _body_once(nc, tc, ps1024, ps512, ones_pad,
               q_d, k_d, v_d, wq_d, wk_d, wv_d, out_d, opt):
    with ExitStack() as body:
        pool_qT = body.enter_context(tc.tile_pool(name="qT", bufs=1))
        pool_kT = body.enter_context(tc.tile_pool(name="kT", bufs=1))
        pool_vaug = body.enter_context(tc.tile_pool(name="vaug", bufs=1))

        qT = [pool_qT.tile([128, S], BF16, name=f"qT{j}") for j in range(NK)]
        kT = [pool_kT.tile([128, S], BF16, name=f"kT{j}") for j in range(NK)]
        # V_aug[p, c, h, 0:128] = V[c*128+p, h*128+j]; [..., 128] = 1.0
        # last dim padded to 132 (264B, 8B-aligned) so every [., c, h, :]
        # moving-operand slice is aligned; col 128 = ones, 129-131 = zero pad
        vaug = pool_vaug.tile([128, NS, H, DH + 4], BF16, name="vaug")

        pool_w = body.enter_context(tc.tile_pool(name="wqk", bufs=opt["w_bufs"]))

        def _load_w(h):
            tiles = {}
            for nm, w_d in (("wq", wq_d), ("wk", wk_d)):
                t = pool_w.tile([128, NK, DH], BF16, name=nm)
                nc.scalar.dma_start(
                    t[:], w_d[:, h * DH:(h + 1) * DH].rearrange(
                        "(kc p) n -> p kc n", p=128))
                tiles[nm] = t
            return tiles

        # ---------------- Phase V: v -> vT -> V -> V_aug (SBUF) -------------
        with ExitStack() as pv:
            pool_vT = pv.enter_context(tc.tile_pool(name="vT", bufs=1))
            pool_wv = pv.enter_context(tc.tile_pool(name="wv", bufs=opt["w_bufs"]))

            vT = [pool_vT.tile([128, S], BF16, name=f"vT{j}") for j in range(NK)]
            for j in range(NK):
                nc.sync.dma_start_transpose(vT[j][:], v_d[:, j * 128:(j + 1) * 128])

            def _load_wv(n):
                t = pool_wv.tile([128, NK, 512], BF16, name="wv")
                src = wv_d[:, n * 512:(n + 1) * 512].rearrange(
                    "(kc p) n -> p kc n", p=128)
                for half in range(2):
                    nq = NK // 2
                    nc.scalar.dma_start(t[:, half * nq:(half + 1) * nq, :],
                                        src[:, half * nq:(half + 1) * nq, :])
                return t

            # prefetch wv0/wv1 and the first heads' W slices ahead of the
            # q/k transposes so the V pipeline is never DMA-starved
            wv_pre = [_load_wv(0), _load_wv(1)]
            w_tiles = _load_w(0)
            w_next = _load_w(1)

            # q/k transposes ride the same queue; they complete during V compute
            for j in range(NK):
                nc.sync.dma_start_transpose(qT[j][:], q_d[:, j * 128:(j + 1) * 128])
            for j in range(NK):
                nc.sync.dma_start_transpose(kT[j][:], k_d[:, j * 128:(j + 1) * 128])

            # ones column of V_aug
            for m in range(NS):
                nc.vector.tensor_copy(vaug[:, m, :, DH:DH + 4], ones_pad[:])

            # V projection, n-slice outer so wv double-buffers cleanly
            for n in range(4):
                wv_t = wv_pre[n] if n < 2 else _load_wv(n)
                for m in range(NS):
                    ps = ps512.tile([128, 512], F32, name="ps512")
                    for jk in range(NK):
                        nc.tensor.matmul(
                            ps[:], vT[jk][:, m * 128:(m + 1) * 128],
                            wv_t[:, jk, :],
                            start=(jk == 0), stop=(jk == NK - 1))
                    # evacuate into V_aug (dest strided over the head dim)
                    nc.vector.tensor_copy(
                        vaug[:, m, n * 4:(n + 1) * 4, 0:DH],
                        ps[:].rearrange("p (h d) -> p h d", d=DH))

        # ---------------- Per-head pipeline ----------------
        with ExitStack() as ph:
            pool_hh = ph.enter_context(tc.tile_pool(name="hh", bufs=opt["qk_bufs"]))
            pool_e = ph.enter_context(tc.tile_pool(name="e", bufs=opt["e_bufs"]))
            pool_rs = ph.enter_context(tc.tile_pool(name="rs", bufs=opt["osb_bufs"]))
            pool_osb = ph.enter_context(tc.tile_pool(name="osb", bufs=opt["osb_bufs"]))
            vstat = opt["pv_mode"] == "v_stat"
            if vstat:
                pool_acc = ph.enter_context(tc.tile_pool(name="acc", bufs=2))
                pool_oT = ph.enter_context(tc.tile_pool(name="oT", bufs=2))
                pool_onat = ph.enter_context(tc.tile_pool(name="onat", bufs=2))
                pool_dsb = ph.enter_context(tc.tile_pool(name="dsb", bufs=2))
                pool_denT = ph.enter_context(tc.tile_pool(name="denT", bufs=2))
                c2 = ph.enter_context(tc.tile_pool(name="c2", bufs=1))
                o16f = c2.tile([128, 16], F32, name="o16f")
                nc.gpsimd.memset(o16f[:], 1.0)
                ones16 = c2.tile([128, 16], BF16, name="ones16")
                nc.vector.tensor_copy(ones16[:], o16f[:])

            def _proj_mms(ps, w_tiles, nm, j0, j1):
                # half-outer: each accumulation group stays within one PSUM
                # bank (consecutive same-bank MMs pipeline drain/fill; bank
                # ping-pong between consecutive MMs measured ~40% slower)
                xT = qT if nm == "wq" else kT
                for half in range(2):
                    for jk in range(j0, j1):
                        nc.tensor.matmul(
                            ps[:, half * 512:(half + 1) * 512],
                            w_tiles[nm][:, jk, :],
                            xT[jk][:, half * 512:(half + 1) * 512],
                            start=(jk == 0), stop=(jk == NK - 1))

            def _proj_evac(ps, nm):
                o = pool_hh.tile([128, S], BF16, name=f"hh_{nm}")
                # two half evacuations: the first scores chunk only waits on
                # the first half instead of the whole [128, 1024] copy
                nc.vector.tensor_copy(o[:, 0:512], ps[:, 0:512])
                nc.vector.tensor_copy(o[:, 512:1024], ps[:, 512:1024])
                return o

            def _proj_one(w_tiles, nm):
                """QT_h or KT_h [128(dh), S] bf16 (32 MMs + 2 half evacs)."""
                ps = ps1024.tile([128, 1024], F32, name="ps1024")
                _proj_mms(ps, w_tiles, nm, 0, NK)
                return _proj_evac(ps, nm)

            def _score_chunk(qh, kh, e_tiles, c, est=None):
                ps = ps1024.tile([128, 1024], F32, name="ps1024")
                for half in range(2):
                    nc.tensor.matmul(
                        ps[:, half * 512:(half + 1) * 512],
                        kh[:, c * 128:(c + 1) * 128],
                        qh[:, half * 512:(half + 1) * 512],
                        start=True, stop=True)
                e_t = pool_e.tile([128, 1024], BF16, name=f"e{c}")
                nc.scalar.activation(e_t[:], ps[:],
                                     mybir.ActivationFunctionType.Exp,
                                     scale=SCALE)
                e_tiles[c] = e_t
                if est is not None:
                    _esum_step(est, e_tiles, c)

            def _esum_step(est, e_tiles, c):
                # running elementwise sum of the e chunks on the idle GPSIMD:
                # two 4-chunk chains, then a final combine; den comes from one
                # ones-matmul on the result instead of 8 per-chunk matmuls
                add = mybir.AluOpType.add
                if c in (1, 5):
                    nm = "accA" if c == 1 else "accB"
                    t = pool_acc.tile([128, 1024], BF16, name=nm)
                    nc.gpsimd.tensor_tensor(t[:], e_tiles[c - 1][:],
                                            e_tiles[c][:], op=add)
                    est[nm] = t
                elif c in (2, 3, 6, 7):
                    nm = "accA" if c < 4 else "accB"
                    t = pool_acc.tile([128, 1024], BF16, name=nm)
                    nc.gpsimd.tensor_tensor(t[:], est[nm][:], e_tiles[c][:],
                                            op=add)
                    est[nm] = t
                    if c == 7:
                        f = pool_acc.tile([128, 1024], BF16, name="accF")
                        nc.gpsimd.tensor_tensor(f[:], est["accA"][:],
                                                est["accB"][:], op=add)
                        est["fin"] = f

            def _pv_vstat_mms(h, e_tiles):
                """out^T accumulation: V chunk stationary, e moving (N=512).
                half-outer keeps each accumulation group in one PSUM bank."""
                psO = ps1024.tile([128, 1024], F32, name="ps1024")
                for half in range(2):
                    for c in range(NS):
                        nc.tensor.matmul(
                            psO[:, half * 512:(half + 1) * 512],
                            vaug[:, c, h, 0:DH],
                            e_tiles[c][:, half * 512:(half + 1) * 512],
                            start=(c == 0), stop=(c == NS - 1))
                oT = pool_oT.tile([128, 1024], BF16, name="oT")
                nc.vector.tensor_copy(oT[:], psO[:])
                onat = pool_onat.tile([128, NS, DH], BF16, name="onat")
                nc.sync.dma_start_transpose(onat[:], oT[:])
                return onat

            def _den_vstat(est):
                """den for all sq of this head: [16,1024] replicated ->
                xbar transpose -> [128, 8(,16)] -> reciprocal [128, 8]."""
                dps = ps512.tile([128, 512], F32, name="ps512")
                dps2 = ps512.tile([128, 512], F32, name="ps512")
                nc.tensor.matmul(dps[0:16, :], ones16[:],
                                 est["fin"][:, 0:512], start=True, stop=True)
                nc.tensor.matmul(dps2[0:16, :], ones16[:],
                                 est["fin"][:, 512:1024], start=True, stop=True)
                dsb = pool_dsb.tile([16, 1024], BF16, name="dsb")
                nc.vector.tensor_copy(dsb[:, 0:512], dps[0:16, :])
                nc.vector.tensor_copy(dsb[:, 512:1024], dps2[0:16, :])
                denT = pool_denT.tile([128, NS, 16], BF16, name="denT")
                nc.sync.dma_start_transpose(denT[:], dsb[:])
                denf = pool_rs.tile([128, NS], F32, name="denf")
                nc.vector.tensor_copy(denf[:], denT[:, :, 0:1])
                rs = pool_rs.tile([128, NS], F32, name="rs8")
                nc.vector.reciprocal_approx_fast(rs[:], denf[:])
                return rs

            def _norm_store_vstat(h, onat, rs, o_sb):
                for t in range(NS):
                    nc.vector.tensor_scalar_mul(o_sb[:, t, :], onat[:, t, :],
                                                rs[:, t:t + 1])
                _store_head(h, o_sb)

            def _pv_tile(h, e_tiles, o_sb, t):
                ps = ps512.tile([128, 512], F32, name="ps512")
                po = ps[:, 0:DH + 4]
                for c in range(NS):
                    nc.tensor.matmul(po, e_tiles[c][:, t * 128:(t + 1) * 128],
                                     vaug[:, c, h, :],
                                     start=(c == 0), stop=(c == NS - 1))
                rs = pool_rs.tile([128, 1], F32, name="rs")
                nc.vector.reciprocal_approx_fast(rs[:], ps[:, DH:DH + 1])
                nc.vector.tensor_scalar_mul(o_sb[:, t, :], ps[:, 0:DH], rs[:])

            def _store_head(h, o_sb):
                nc.gpsimd.dma_start(
                    out_d[:, h * DH:(h + 1) * DH].rearrange(
                        "(t p) d -> p t d", p=128),
                    o_sb[:])

            # Shifted pipeline: iteration h runs PV(h) interleaved with
            # scores(h+1), then emits proj(h+2) as ACT-hiding filler.
            # Prologue: proj(0) + proj(1), with scores(0) spaced by proj(1)
            # pieces so ACT exp keeps up.
            q0 = _proj_one(w_tiles, "wq")
            k0 = _proj_one(w_tiles, "wk")
            e_cur = [None] * NS
            est_cur = {} if vstat else None
            _score_chunk(q0, k0, e_cur, 0, est_cur)
            _score_chunk(q0, k0, e_cur, 1, est_cur)
            q1 = _proj_one(w_next, "wq")
            _score_chunk(q0, k0, e_cur, 2, est_cur)
            _score_chunk(q0, k0, e_cur, 3, est_cur)
            k1 = _proj_one(w_next, "wk")
            for c in range(4, NS):
                _score_chunk(q0, k0, e_cur, c, est_cur)
            cur = (q1, k1)
            w_next = _load_w(2)

            for h in range(H):
                o_sb = pool_osb.tile([128, NS, DH], F32, name="osb")
                e_nxt = [None] * NS
                est_nxt = {} if vstat else None
                if vstat:
                    onat = _pv_vstat_mms(h, e_cur)
                    if h + 1 < H:
                        _score_chunk(cur[0], cur[1], e_nxt, 0, est_nxt)
                        _score_chunk(cur[0], cur[1], e_nxt, 1, est_nxt)
                        if h + 2 < H:
                            ps_q = ps1024.tile([128, 1024], F32, name="ps1024")
                            _proj_mms(ps_q, w_next, "wq", 0, 6)
                            rs = _den_vstat(est_cur)
                            _score_chunk(cur[0], cur[1], e_nxt, 2, est_nxt)
                            _proj_mms(ps_q, w_next, "wq", 6, 11)
                            _score_chunk(cur[0], cur[1], e_nxt, 3, est_nxt)
                            _proj_mms(ps_q, w_next, "wq", 11, 16)
                            nq = _proj_evac(ps_q, "wq")
                            ps_k = ps1024.tile([128, 1024], F32, name="ps1024")
                            _proj_mms(ps_k, w_next, "wk", 0, 4)
                            _score_chunk(cur[0], cur[1], e_nxt, 4, est_nxt)
                            _proj_mms(ps_k, w_next, "wk", 4, 8)
                            _score_chunk(cur[0], cur[1], e_nxt, 5, est_nxt)
                            _proj_mms(ps_k, w_next, "wk", 8, 12)
                            _score_chunk(cur[0], cur[1], e_nxt, 6, est_nxt)
                            _proj_mms(ps_k, w_next, "wk", 12, 16)
                            nk = _proj_evac(ps_k, "wk")
                            _score_chunk(cur[0], cur[1], e_nxt, 7, est_nxt)
                            nxt = (nq, nk)
                            w_next = _load_w(h + 3) if h + 3 < H else None
                        else:
                            rs = _den_vstat(est_cur)
                            for c in range(2, NS):
                                _score_chunk(cur[0], cur[1], e_nxt, c, est_nxt)
                            nxt = None
                    else:
                        rs = _den_vstat(est_cur)
                        nxt = None
                    _norm_store_vstat(h, onat, rs, o_sb)
                    cur = nxt if nxt is not None else cur
                    e_cur, est_cur = e_nxt, est_nxt
                    continue
                if h + 1 < H:
                    # 3 PV tiles up front cover the QT/KT evac latency of the
                    # upcoming scores; the next 5 interleave 1:1; the last two
                    # scores chunks weave between proj(h+2) halves so ACT exp
                    # never gates the PE
                    _pv_tile(h, e_cur, o_sb, 0)
                    _pv_tile(h, e_cur, o_sb, 1)
                    _pv_tile(h, e_cur, o_sb, 2)
                    for c in range(NS - 2):
                        _score_chunk(cur[0], cur[1], e_nxt, c)
                        if 3 + c < NS:
                            _pv_tile(h, e_cur, o_sb, 3 + c)
                    if h + 2 < H:
                        ps_q = ps1024.tile([128, 1024], F32, name="ps1024")
                        _proj_mms(ps_q, w_next, "wq", 0, NK)
                        nq = _proj_evac(ps_q, "wq")
                        _score_chunk(cur[0], cur[1], e_nxt, NS - 2)
                        ps_k = ps1024.tile([128, 1024], F32, name="ps1024")
                        _proj_mms(ps_k, w_next, "wk", 0, NK // 2)
                        _score_chunk(cur[0], cur[1], e_nxt, NS - 1)
                        _proj_mms(ps_k, w_next, "wk", NK // 2, NK)
                        nk = _proj_evac(ps_k, "wk")
                        nxt = (nq, nk)
                        w_next = _load_w(h + 3) if h + 3 < H else None
                    else:
                        _score_chunk(cur[0], cur[1], e_nxt, NS - 2)
                        _score_chunk(cur[0], cur[1], e_nxt, NS - 1)
                        nxt = None
                else:
                    for t in range(NS):
                        _pv_tile(h, e_cur, o_sb, t)
                    nxt = None
                _store_head(h, o_sb)
                cur, e_cur = nxt if nxt is not None else cur, e_nxt


def _make_runner(nc, n_cores):
    """Jitted SPMD runner (q/k/v sharded over cores, weights replicated)."""
    import jax
    from jax.sharding import Mesh, PartitionSpec
    from jax.experimental.shard_map import shard_map
    from concourse import bass2jax
    from concourse.bass2jax import _bass_exec_p, install_neuronx_cc_hook

    install_neuronx_cc_hook()
    partition_name = nc.partition_id_tensor.name if nc.partition_id_tensor else None
    in_names, out_names, out_avals, zero_outs = [], [], [], []
    for alloc in nc.m.functions[0].allocations:
        if not isinstance(alloc, mybir.MemoryLocationSet):
            continue
        name = alloc.memorylocations[0].name
        if alloc.kind == "ExternalInput":
            if name != partition_name:
                in_names.append(name)
        elif alloc.kind == "ExternalOutput":
            out_names.append(name)
            shape = tuple(alloc.tensor_shape)
            dtype = mybir.dt.np(alloc.dtype)
            out_avals.append(jax.core.ShapedArray(shape, dtype))
            zero_outs.append(np.zeros(shape, dtype))
    sharded_in = {"q", "k", "v"}
    in_names_all = in_names + out_names
    if partition_name is not None:
        in_names_all.append(partition_name)

    def _body(*args):
        operands = list(args)
        if partition_name is not None:
            operands.append(bass2jax.partition_id_tensor())
        outs = _bass_exec_p.bind(
            *operands,
            out_avals=tuple(out_avals),
            in_names=tuple(in_names_all),
            out_names=tuple(out_names),
            lowering_input_output_aliases=(),
            sim_require_finite=True,
            sim_require_nnan=True,
            nc=nc,
        )
        return tuple(outs)

    devices = jax.devices()[:n_cores]
    mesh = Mesh(np.asarray(devices), ("core",))
    in_specs = tuple(
        PartitionSpec("core") if n in sharded_in else PartitionSpec()
        for n in in_names
    ) + (PartitionSpec("core"),) * len(out_names)
    out_specs = (PartitionSpec("core"),) * len(out_names)
    jitted = jax.jit(
        shard_map(_body, mesh=mesh, in_specs=in_specs, out_specs=out_specs,
                  check_rep=False),
        keep_unused=True,
    )

    def run(shared_map_, per_core_maps):
        import jax as _jax
        args = []
        for n in in_names:
            if n in sharded_in:
                args.append(np.concatenate([m[n] for m in per_core_maps], axis=0))
            else:
                args.append(shared_map_[n])
        concat_zeros = [
            np.zeros((n_cores * z.shape[0], *z.shape[1:]), z.dtype) for z in zero_outs
        ]
        out_arrs = jitted(*args, *concat_zeros)
        _jax.block_until_ready(out_arrs)
        return [
            {
                name: np.asarray(out_arrs[i]).reshape(n_cores, *out_avals[i].shape)[c]
                for i, name in enumerate(out_names)
            }
            for c in range(n_cores)
        ]

    return run


def _to_bf16(x):
    import ml_dtypes
    return np.asarray(x, dtype=np.float32).astype(ml_dtypes.bfloat16)


def _get_compiled():
    if "run" not in _CACHE:
        nc = build()
        _CACHE["run"] = _make_runner(nc, B)
    return _CACHE["run"]


def kernel(q, k, v, Wq, Wk, Wv):
    run = _get_compiled()
    shared = {"Wq": _to_bf16(Wq), "Wk": _to_bf16(Wk), "Wv": _to_bf16(Wv)}
    q = _to_bf16(q)
    k = _to_bf16(k)
    v = _to_bf16(v)
    per_core = [{"q": q[b], "k": k[b], "v": v[b]} for b in range(B)]
    results = run(shared, per_core)
    out = np.stack([results[b]["out"] for b in range(B)], axis=0)
    return out.astype(np.float32)


if __name__ == "__main__":
    rng = np.random.default_rng(0)
    qq = rng.standard_normal((B, S, D), dtype=np.float32)
    kk = rng.standard_normal((B, S, D), dtype=np.float32)
    vv = rng.standard_normal((B, S, D), dtype=np.float32)
    sc = np.float32(1.0 / np.sqrt(D))
    Wq = rng.standard_normal((D, D), dtype=np.float32) * sc
    Wk = rng.standard_normal((D, D), dtype=np.float32) * sc
    Wv = rng.standard_normal((D, D), dtype=np.float32) * sc
    o = kernel(q=qq, k=kk, v=vv, Wq=Wq, Wk=Wk, Wv=Wv)
    print("out", o.shape, o.dtype, np.abs(o).max())


# revision 23
# speedup vs baseline: 1.4640x; 1.4577x over previous
"""Multi-head attention (B=8, S=1024, D=2048, H=16) on 8 Trainium2 NeuronCores.

Sharding: pure data parallel — core b computes batch element b. Weights are
replicated. All matmuls in bf16 (1 cyc/row on PE, same as f32r, but with FWL
weight loads and half the DMA/SBUF), accumulation in fp32 PSUM.

Structure per core:
  - q/k/v are transposed by the DMA xbar (dma_start_transpose) straight into
    SBUF as xT chunk tiles [128, S] — zero PE transpose cost.
  - V = v@Wv staged in SBUF augmented with a ones column per chunk:
    V_aug[:, c, h, 0:128] = V chunk, V_aug[:, c, h, 128] = 1.
  - Per head h: project QT_h/KT_h (W chunk stationary, xT moving), scores
    (KT chunk stationary), exp on ACT -> e chunks [sk, sq] bf16, then PV with
    the e chunks STATIONARY and V_aug moving:
        out[sq, 0:129] = sum_c e_c[:, sq-tile]^T @ V_aug[:, c, h, :]
    Column 128 is the softmax denominator in natural per-partition layout, so
    normalization is one reciprocal + one per-partition tensor_scalar mul.
    No tail transposes, no separate denominator matmuls.
  - PV(h) matmuls are interleaved into the scores(h+1) stream so the PE fills
    the gaps where scores wait on ACT draining PSUM.

Self-contained: builds the Bass program, shards inputs, runs SPMD via PJRT,
reassembles the full output.
"""
import numpy as np
from contextlib import ExitStack

import concourse.bacc as bacc
import concourse.mybir as mybir
import concourse.tile as tile

B, S, D, H = 8, 1024, 2048, 16
DH = D // H            # 128
NK = D // 128          # 16 k-chunks
NS = S // 128          # 8 s-tiles
F32 = mybir.dt.float32
BF16 = mybir.dt.bfloat16
SCALE = 1.0 / float(np.sqrt(DH))

_CACHE = {}


def build(opt=None, reps=1, timing=False):
    _defaults = dict(
        ps1024_bufs=3,
        ps512_bufs=2,
        e_bufs=2,
        w_bufs=2,
        qk_bufs=2,
        osb_bufs=2,
        sc_pv_pattern=1,   # interleave PV tiles between scores chunks
        pv_mode="e_stat",  # "e_stat": e-chunk stationary, den via ones column
                           # "v_stat": V-chunk stationary (N=512 moving), den
                           #   via gpsimd E_sum + ones-matmul, out^T via xbar
    )
    _defaults.update(opt or {})
    opt = _defaults
    nc = bacc.Bacc("TRN2", target_bir_lowering=False, debug=False)

    def _in(name, shape, dt_):
        if timing:
            return nc.dram_tensor(name, shape, dt_).ap()
        return nc.dram_tensor(name, shape, dt_, kind="ExternalInput").ap()

    q_d = _in("q", [S, D], BF16)
    k_d = _in("k", [S, D], BF16)
    v_d = _in("v", [S, D], BF16)
    wq_d = _in("Wq", [D, D], BF16)
    wk_d = _in("Wk", [D, D], BF16)
    wv_d = _in("Wv", [D, D], BF16)
    if timing:
        out_d = nc.dram_tensor("out", [S, D], F32).ap()
        tout_d = nc.dram_tensor("tout", [1, 8], F32, kind="ExternalOutput").ap()
    else:
        out_d = nc.dram_tensor("out", [S, D], F32, kind="ExternalOutput").ap()

    with tile.TileContext(nc) as tc, ExitStack() as ctx:
        ps1024 = ctx.enter_context(
            tc.tile_pool(name="ps1024", bufs=opt["ps1024_bufs"], space="PSUM"))
        ps512 = ctx.enter_context(
            tc.tile_pool(name="ps512", bufs=opt["ps512_bufs"], space="PSUM"))
        const = ctx.enter_context(tc.tile_pool(name="const", bufs=1))

        ones_f = const.tile([128, 16], F32, name="ones_f")
        nc.gpsimd.memset(ones_f[:], 1.0)
        zeros_f = const.tile([128, 16, 3], F32, name="zeros_f")
        nc.gpsimd.memset(zeros_f[:], 0.0)
        ones_pad = const.tile([128, 16, 4], BF16, name="ones_pad")
        # col 0 = 1.0 (the den ones column), cols 1-3 = alignment pad zeros;
        # memset on a strided view is unreliable, so build via two copies
        nc.vector.tensor_copy(ones_pad[:, :, 0:1], ones_f[:])
        nc.vector.tensor_copy(ones_pad[:, :, 1:4], zeros_f[:])

        if timing:
            with tc.tile_pool(name="zfill", bufs=1) as zpool:
                zf = zpool.tile([128, D], F32, name="zfill")
                nc.gpsimd.memset(zf[:], 0.0)
                zr = zpool.tile([128, D], BF16, name="zfill_b")
                nc.vector.tensor_copy(zr[:], zf[:])
                for x in (q_d, k_d, v_d, wq_d, wk_d, wv_d):
                    nrow = x.shape[0] // 128
                    for i in range(nrow):
                        nc.sync.dma_start(x[i * 128:(i + 1) * 128, :], zr[:])

        for _rep in range(reps):
            _body_once(nc, tc, ps1024, ps512, ones_pad,
                       q_d, k_d, v_d, wq_d, wk_d, wv_d, out_d, opt)
        if timing:
            zo = const.tile([1, 8], F32, name="zo")
            nc.gpsimd.memset(zo[:], 0.0)
            nc.sync.dma_start(tout_d[:], zo[:])

    nc.compile()
    return nc


def _body_once(nc, tc, ps1024, ps512, ones_pad,
               q_d, k_d, v_d, wq_d, wk_d, wv_d, out_d, opt):
    with ExitStack() as body:
        pool_qT = body.enter_context(tc.tile_pool(name="qT", bufs=1))
        pool_kT = body.enter_context(tc.tile_pool(name="kT", bufs=1))
        pool_vaug = body.enter_context(tc.tile_pool(name="vaug", bufs=1))

        qT = [pool_qT.tile([128, S], BF16, name=f"qT{j}") for j in range(NK)]
        kT = [pool_kT.tile([128, S], BF16, name=f"kT{j}") for j in range(NK)]
        # V_aug[p, c, h, 0:128] = V[c*128+p, h*128+j]; [..., 128] = 1.0
        # last dim padded to 132 (264B, 8B-aligned) so every [., c, h, :]
        # moving-operand slice is aligned; col 128 = ones, 129-131 = zero pad
        vaug = pool_vaug.tile([128, NS, H, DH + 4], BF16, name="vaug")

        pool_w = body.enter_context(tc.tile_pool(name="wqk", bufs=opt["w_bufs"]))

        def _load_w(h):
            tiles = {}
            for nm, w_d in (("wq", wq_d), ("wk", wk_d)):
                t = pool_w.tile([128, NK, DH], BF16, name=nm)
                nc.scalar.dma_start(
                    t[:], w_d[:, h * DH:(h + 1) * DH].rearrange(
                        "(kc p) n -> p kc n", p=128))
                tiles[nm] = t
            return tiles

        # ---------------- Phase V: v -> vT -> V -> V_aug (SBUF) -------------
        with ExitStack() as pv:
            pool_vT = pv.enter_context(tc.tile_pool(name="vT", bufs=1))
            pool_wv = pv.enter_context(tc.tile_pool(name="wv", bufs=opt["w_bufs"]))

            vT = [pool_vT.tile([128, S], BF16, name=f"vT{j}") for j in range(NK)]
            for j in range(NK):
                nc.sync.dma_start_transpose(vT[j][:], v_d[:, j * 128:(j + 1) * 128])

            def _load_wv(n):
                t = pool_wv.tile([128, NK, 512], BF16, name="wv")
                src = wv_d[:, n * 512:(n + 1) * 512].rearrange(
                    "(kc p) n -> p kc n", p=128)
                for half in range(2):
                    nq = NK // 2
                    nc.scalar.dma_start(t[:, half * nq:(half + 1) * nq, :],
                                        src[:, half * nq:(half + 1) * nq, :])
                return t

            # prefetch wv0/wv1 and the first heads' W slices ahead of the
            # q/k transposes so the V pipeline is never DMA-starved
            wv_pre = [_load_wv(0), _load_wv(1)]
            w_tiles = _load_w(0)
            w_next = _load_w(1)

            # q/k transposes ride the same queue; they complete during V compute
            for j in range(NK):
                nc.sync.dma_start_transpose(qT[j][:], q_d[:, j * 128:(j + 1) * 128])
            for j in range(NK):
                nc.sync.dma_start_transpose(kT[j][:], k_d[:, j * 128:(j + 1) * 128])

            # ones column of V_aug
            for m in range(NS):
                nc.vector.tensor_copy(vaug[:, m, :, DH:DH + 4], ones_pad[:])

            # V projection, n-slice outer so wv double-buffers cleanly
            for n in range(4):
                wv_t = wv_pre[n] if n < 2 else _load_wv(n)
                for m in range(NS):
                    ps = ps512.tile([128, 512], F32, name="ps512")
                    for jk in range(NK):
                        nc.tensor.matmul(
                            ps[:], vT[jk][:, m * 128:(m + 1) * 128],
                            wv_t[:, jk, :],
                            start=(jk == 0), stop=(jk == NK - 1))
                    # evacuate into V_aug (dest strided over the head dim)
                    nc.vector.tensor_copy(
                        vaug[:, m, n * 4:(n + 1) * 4, 0:DH],
                        ps[:].rearrange("p (h d) -> p h d", d=DH))

        # ---------------- Per-head pipeline ----------------
        with ExitStack() as ph:
            pool_hh = ph.enter_context(tc.tile_pool(name="hh", bufs=opt["qk_bufs"]))
            pool_e = ph.enter_context(tc.tile_pool(name="e", bufs=opt["e_bufs"]))
            pool_rs = ph.enter_context(tc.tile_pool(name="rs", bufs=opt["osb_bufs"]))
            pool_osb = ph.enter_context(tc.tile_pool(name="osb", bufs=opt["osb_bufs"]))
            vstat = opt["pv_mode"] == "v_stat"
            if vstat:
                pool_acc = ph.enter_context(tc.tile_pool(name="acc", bufs=2))
                pool_oT = ph.enter_context(tc.tile_pool(name="oT", bufs=2))
                pool_onat = ph.enter_context(tc.tile_pool(name="onat", bufs=2))
                pool_dsb = ph.enter_context(tc.tile_pool(name="dsb", bufs=2))
                pool_denT = ph.enter_context(tc.tile_pool(name="denT", bufs=2))
                c2 = ph.enter_context(tc.tile_pool(name="c2", bufs=1))
                o16f = c2.tile([128, 16], F32, name="o16f")
                nc.gpsimd.memset(o16f[:], 1.0)
                ones16 = c2.tile([128, 16], BF16, name="ones16")
                nc.vector.tensor_copy(ones16[:], o16f[:])

            def _proj_mms(ps, w_tiles, nm, j0, j1):
                # half-inner: consecutive MMs alternate PSUM banks, letting
                # MM(i+1) fill while MM(i) drains (same-bank runs measured
                # ~150us slower end-to-end) and reusing each stationary W
                # chunk for both halves
                xT = qT if nm == "wq" else kT
                for jk in range(j0, j1):
                    for half in range(2):
                        nc.tensor.matmul(
                            ps[:, half * 512:(half + 1) * 512],
                            w_tiles[nm][:, jk, :],
                            xT[jk][:, half * 512:(half + 1) * 512],
                            start=(jk == 0), stop=(jk == NK - 1))

            def _proj_evac(ps, nm):
                o = pool_hh.tile([128, S], BF16, name=f"hh_{nm}")
                # two half evacuations: the first scores chunk only waits on
                # the first half instead of the whole [128, 1024] copy
                nc.vector.tensor_copy(o[:, 0:512], ps[:, 0:512])
                nc.vector.tensor_copy(o[:, 512:1024], ps[:, 512:1024])
                return o

            def _proj_one(w_tiles, nm):
                """QT_h or KT_h [128(dh), S] bf16 (32 MMs + 2 half evacs)."""
                ps = ps1024.tile([128, 1024], F32, name="ps1024")
                _proj_mms(ps, w_tiles, nm, 0, NK)
                return _proj_evac(ps, nm)

            def _score_chunk(qh, kh, e_tiles, c, est=None):
                ps = ps1024.tile([128, 1024], F32, name="ps1024")
                for half in range(2):
                    nc.tensor.matmul(
                        ps[:, half * 512:(half + 1) * 512],
                        kh[:, c * 128:(c + 1) * 128],
                        qh[:, half * 512:(half + 1) * 512],
                        start=True, stop=True)
                e_t = pool_e.tile([128, 1024], BF16, name=f"e{c}")
                nc.scalar.activation(e_t[:], ps[:],
                                     mybir.ActivationFunctionType.Exp,
                                     scale=SCALE)
                e_tiles[c] = e_t
                if est is not None:
                    _esum_step(est, e_tiles, c)

            def _esum_step(est, e_tiles, c):
                # running elementwise sum of the e chunks on the idle GPSIMD:
                # two 4-chunk chains, then a final combine; den comes from one
                # ones-matmul on the result instead of 8 per-chunk matmuls
                add = mybir.AluOpType.add
                if c in (1, 5):
                    nm = "accA" if c == 1 else "accB"
                    t = pool_acc.tile([128, 1024], BF16, name=nm)
                    nc.gpsimd.tensor_tensor(t[:], e_tiles[c - 1][:],
                                            e_tiles[c][:], op=add)
                    est[nm] = t
                elif c in (2, 3, 6, 7):
                    nm = "accA" if c < 4 else "accB"
                    t = pool_acc.tile([128, 1024], BF16, name=nm)
                    nc.gpsimd.tensor_tensor(t[:], est[nm][:], e_tiles[c][:],
                                            op=add)
                    est[nm] = t
                    if c == 7:
                        f = pool_acc.tile([128, 1024], BF16, name="accF")
                        nc.gpsimd.tensor_tensor(f[:], est["accA"][:],
                                                est["accB"][:], op=add)
                        est["fin"] = f

            def _pv_vstat_mms(h, e_tiles):
                """out^T accumulation: V chunk stationary, e moving (N=512).
                half-outer keeps each accumulation group in one PSUM bank."""
                psO = ps1024.tile([128, 1024], F32, name="ps1024")
                for c in range(NS):
                    for half in range(2):
                        nc.tensor.matmul(
                            psO[:, half * 512:(half + 1) * 512],
                            vaug[:, c, h, 0:DH],
                            e_tiles[c][:, half * 512:(half + 1) * 512],
                            start=(c == 0), stop=(c == NS - 1))
                oT = pool_oT.tile([128, 1024], BF16, name="oT")
                nc.vector.tensor_copy(oT[:], psO[:])
                onat = pool_onat.tile([128, NS, DH], BF16, name="onat")
                nc.sync.dma_start_transpose(onat[:], oT[:])
                return onat

            def _den_vstat(est):
                """den for all sq of this head: [16,1024] replicated ->
                xbar transpose -> [128, 8(,16)] -> reciprocal [128, 8]."""
                dps = ps512.tile([128, 512], F32, name="ps512")
                dps2 = ps512.tile([128, 512], F32, name="ps512")
                nc.tensor.matmul(dps[0:16, :], ones16[:],
                                 est["fin"][:, 0:512], start=True, stop=True)
                nc.tensor.matmul(dps2[0:16, :], ones16[:],
                                 est["fin"][:, 512:1024], start=True, stop=True)
                dsb = pool_dsb.tile([16, 1024], BF16, name="dsb")
                nc.vector.tensor_copy(dsb[:, 0:512], dps[0:16, :])
                nc.vector.tensor_copy(dsb[:, 512:1024], dps2[0:16, :])
                denT = pool_denT.tile([128, NS, 16], BF16, name="denT")
                nc.sync.dma_start_transpose(denT[:], dsb[:])
                denf = pool_rs.tile([128, NS], F32, name="denf")
                nc.vector.tensor_copy(denf[:], denT[:, :, 0:1])
                rs = pool_rs.tile([128, NS], F32, name="rs8")
                nc.vector.reciprocal_approx_fast(rs[:], denf[:])
                return rs

            def _norm_store_vstat(h, onat, rs, o_sb):
                for t in range(NS):
                    nc.vector.tensor_scalar_mul(o_sb[:, t, :], onat[:, t, :],
                                                rs[:, t:t + 1])
                _store_head(h, o_sb)

            def _pv_tile(h, e_tiles, o_sb, t):
                ps = ps512.tile([128, 512], F32, name="ps512")
                po = ps[:, 0:DH + 4]
                for c in range(NS):
                    nc.tensor.matmul(po, e_tiles[c][:, t * 128:(t + 1) * 128],
                                     vaug[:, c, h, :],
                                     start=(c == 0), stop=(c == NS - 1))
                rs = pool_rs.tile([128, 1], F32, name="rs")
                nc.vector.reciprocal_approx_fast(rs[:], ps[:, DH:DH + 1])
                nc.vector.tensor_scalar_mul(o_sb[:, t, :], ps[:, 0:DH], rs[:])

            def _store_head(h, o_sb):
                nc.gpsimd.dma_start(
                    out_d[:, h * DH:(h + 1) * DH].rearrange(
                        "(t p) d -> p t d", p=128),
                    o_sb[:])

            # Shifted pipeline: iteration h runs PV(h) interleaved with
            # scores(h+1), then emits proj(h+2) as ACT-hiding filler.
            # Prologue: proj(0) + proj(1), with scores(0) spaced by proj(1)
            # pieces so ACT exp keeps up.
            q0 = _proj_one(w_tiles, "wq")
            k0 = _proj_one(w_tiles, "wk")
            e_cur = [None] * NS
            est_cur = {} if vstat else None
            _score_chunk(q0, k0, e_cur, 0, est_cur)
            _score_chunk(q0, k0, e_cur, 1, est_cur)
            q1 = _proj_one(w_next, "wq")
            _score_chunk(q0, k0, e_cur, 2, est_cur)
            _score_chunk(q0, k0, e_cur, 3, est_cur)
            k1 = _proj_one(w_next, "wk")
            for c in range(4, NS):
                _score_chunk(q0, k0, e_cur, c, est_cur)
            cur = (q1, k1)
            w_next = _load_w(2)

            for h in range(H):
                o_sb = pool_osb.tile([128, NS, DH], F32, name="osb")
                e_nxt = [None] * NS
                est_nxt = {} if vstat else None
                if vstat:
                    onat = _pv_vstat_mms(h, e_cur)
                    if h + 1 < H:
                        _score_chunk(cur[0], cur[1], e_nxt, 0, est_nxt)
                        _score_chunk(cur[0], cur[1], e_nxt, 1, est_nxt)
                        if h + 2 < H:
                            ps_q = ps1024.tile([128, 1024], F32, name="ps1024")
                            _proj_mms(ps_q, w_next, "wq", 0, 6)
                            rs = _den_vstat(est_cur)
                            _score_chunk(cur[0], cur[1], e_nxt, 2, est_nxt)
                            _proj_mms(ps_q, w_next, "wq", 6, 11)
                            _score_chunk(cur[0], cur[1], e_nxt, 3, est_nxt)
                            _proj_mms(ps_q, w_next, "wq", 11, 16)
                            nq = _proj_evac(ps_q, "wq")
                            ps_k = ps1024.tile([128, 1024], F32, name="ps1024")
                            _proj_mms(ps_k, w_next, "wk", 0, 4)
                            _score_chunk(cur[0], cur[1], e_nxt, 4, est_nxt)
                            _proj_mms(ps_k, w_next, "wk", 4, 8)
                            _score_chunk(cur[0], cur[1], e_nxt, 5, est_nxt)
                            _proj_mms(ps_k, w_next, "wk", 8, 12)
                            _score_chunk(cur[0], cur[1], e_nxt, 6, est_nxt)
                            _proj_mms(ps_k, w_next, "wk", 12, 16)
                            nk = _proj_evac(ps_k, "wk")
                            _score_chunk(cur[0], cur[1], e_nxt, 7, est_nxt)
                            nxt = (nq, nk)
                            w_next = _load_w(h + 3) if h + 3 < H else None
                        else:
                            rs = _den_vstat(est_cur)
                            for c in range(2, NS):
                                _score_chunk(cur[0], cur[1], e_nxt, c, est_nxt)
                            nxt = None
                    else:
                        rs = _den_vstat(est_cur)
                        nxt = None
                    _norm_store_vstat(h, onat, rs, o_sb)
                    cur = nxt if nxt is not None else cur
                    e_cur, est_cur = e_nxt, est_nxt
                    continue
                if h + 1 < H:
                    # 3 PV tiles up front cover the QT/KT evac latency of the
                    # upcoming scores; the next 5 interleave 1:1; the last two
                    # scores chunks weave between proj(h+2) halves so ACT exp
                    # never gates the PE
                    _pv_tile(h, e_cur, o_sb, 0)
                    _pv_tile(h, e_cur, o_sb, 1)
                    _pv_tile(h, e_cur, o_sb, 2)
                    for c in range(NS - 2):
                        _score_chunk(cur[0], cur[1], e_nxt, c)
                        if 3 + c < NS:
                            _pv_tile(h, e_cur, o_sb, 3 + c)
                    if h + 2 < H:
                        ps_q = ps1024.tile([128, 1024], F32, name="ps1024")
                        _proj_mms(ps_q, w_next, "wq", 0, NK)
                        nq = _proj_evac(ps_q, "wq")
                        _score_chunk(cur[0], cur[1], e_nxt, NS - 2)
                        ps_k = ps1024.tile([128, 1024], F32, name="ps1024")
                        _proj_mms(ps_k, w_next, "wk", 0, NK // 2)
                        _score_chunk(cur[0], cur[1], e_nxt, NS - 1)
                        _proj_mms(ps_k, w_next, "wk", NK // 2, NK)
                        nk = _proj_evac(ps_k, "wk")
                        nxt = (nq, nk)
                        w_next = _load_w(h + 3) if h + 3 < H else None
                    else:
                        _score_chunk(cur[0], cur[1], e_nxt, NS - 2)
                        _score_chunk(cur[0], cur[1], e_nxt, NS - 1)
                        nxt = None
                else:
                    for t in range(NS):
                        _pv_tile(h, e_cur, o_sb, t)
                    nxt = None
                _store_head(h, o_sb)
                cur, e_cur = nxt if nxt is not None else cur, e_nxt


def _make_runner(nc, n_cores):
    """Jitted SPMD runner (q/k/v sharded over cores, weights replicated)."""
    import jax
    from jax.sharding import Mesh, PartitionSpec
    from jax.experimental.shard_map import shard_map
    from concourse import bass2jax
    from concourse.bass2jax import _bass_exec_p, install_neuronx_cc_hook

    install_neuronx_cc_hook()
    partition_name = nc.partition_id_tensor.name if nc.partition_id_tensor else None
    in_names, out_names, out_avals, zero_outs = [], [], [], []
    for alloc in nc.m.functions[0].allocations:
        if not isinstance(alloc, mybir.MemoryLocationSet):
            continue
        name = alloc.memorylocations[0].name
        if alloc.kind == "ExternalInput":
            if name != partition_name:
                in_names.append(name)
        elif alloc.kind == "ExternalOutput":
            out_names.append(name)
            shape = tuple(alloc.tensor_shape)
            dtype = mybir.dt.np(alloc.dtype)
            out_avals.append(jax.core.ShapedArray(shape, dtype))
            zero_outs.append(np.zeros(shape, dtype))
    sharded_in = {"q", "k", "v"}
    in_names_all = in_names + out_names
    if partition_name is not None:
        in_names_all.append(partition_name)

    def _body(*args):
        operands = list(args)
        if partition_name is not None:
            operands.append(bass2jax.partition_id_tensor())
        outs = _bass_exec_p.bind(
            *operands,
            out_avals=tuple(out_avals),
            in_names=tuple(in_names_all),
            out_names=tuple(out_names),
            lowering_input_output_aliases=(),
            sim_require_finite=True,
            sim_require_nnan=True,
            nc=nc,
        )
        return tuple(outs)

    devices = jax.devices()[:n_cores]
    mesh = Mesh(np.asarray(devices), ("core",))
    in_specs = tuple(
        PartitionSpec("core") if n in sharded_in else PartitionSpec()
        for n in in_names
    ) + (PartitionSpec("core"),) * len(out_names)
    out_specs = (PartitionSpec("core"),) * len(out_names)
    jitted = jax.jit(
        shard_map(_body, mesh=mesh, in_specs=in_specs, out_specs=out_specs,
                  check_rep=False),
        keep_unused=True,
    )

    def run(shared_map_, per_core_maps):
        import jax as _jax
        args = []
        for n in in_names:
            if n in sharded_in:
                args.append(np.concatenate([m[n] for m in per_core_maps], axis=0))
            else:
                args.append(shared_map_[n])
        concat_zeros = [
            np.zeros((n_cores * z.shape[0], *z.shape[1:]), z.dtype) for z in zero_outs
        ]
        out_arrs = jitted(*args, *concat_zeros)
        _jax.block_until_ready(out_arrs)
        return [
            {
                name: np.asarray(out_arrs[i]).reshape(n_cores, *out_avals[i].shape)[c]
                for i, name in enumerate(out_names)
            }
            for c in range(n_cores)
        ]

    return run


def _to_bf16(x):
    import ml_dtypes
    return np.asarray(x, dtype=np.float32).astype(ml_dtypes.bfloat16)


def _get_compiled():
    if "run" not in _CACHE:
        nc = build()
        _CACHE["run"] = _make_runner(nc, B)
    return _CACHE["run"]


def kernel(q, k, v, Wq, Wk, Wv):
    run = _get_compiled()
    shared = {"Wq": _to_bf16(Wq), "Wk": _to_bf16(Wk), "Wv": _to_bf16(Wv)}
    q = _to_bf16(q)
    k = _to_bf16(k)
    v = _to_bf16(v)
    per_core = [{"q": q[b], "k": k[b], "v": v[b]} for b in range(B)]
    results = run(shared, per_core)
    out = np.stack([results[b]["out"] for b in range(B)], axis=0)
    return out.astype(np.float32)


if __name__ == "__main__":
    rng = np.random.default_rng(0)
    qq = rng.standard_normal((B, S, D), dtype=np.float32)
    kk = rng.standard_normal((B, S, D), dtype=np.float32)
    vv = rng.standard_normal((B, S, D), dtype=np.float32)
    sc = np.float32(1.0 / np.sqrt(D))
    Wq = rng.standard_normal((D, D), dtype=np.float32) * sc
    Wk = rng.standard_normal((D, D), dtype=np.float32) * sc
    Wv = rng.standard_normal((D, D), dtype=np.float32) * sc
    o = kernel(q=qq, k=kk, v=vv, Wq=Wq, Wk=Wk, Wv=Wv)
    print("out", o.shape, o.dtype, np.abs(o).max())
